# revision 21
# baseline (speedup 1.0000x reference)
"""CrystalGraphConv (CGCNN conv layer) Trainium2 kernel, 8-core data-parallel.

Strategy
--------
Data-parallel over batch B (512 -> 64 per core). Per core, everything is
computed in a feature-major ("transposed") SBUF layout so the gated-conv
matmul z = [atom_exp | gathered | nbr] @ W runs with W as the stationary
operand and the concatenated features streaming as the moving operand:

  pass 1 (streams all inputs from HBM exactly once):
    - mask int32 -> f16 via cast-DMA, transposed on TensorE (via identity
      matmul) to m^T[128 feat, rows]
    - neighbor gather done as a one-hot matmul per batch:
        gathered^T = atom_b.T @ onehot^T, onehot^T[n, r] = (idx[r] == n)
    - atom_exp term uses a free-axis broadcast AP (each column x12)
    - z^T accumulated in PSUM over two K-groups (W rows 0:128, 128:169 +
      a ones-row carrying gc_bias)
    - w = z * m  (masked preactivation) cached in SBUF as f16 [128, rows];
      fused reduce accumulates S1 = sum(w); ACT engine accumulates
      Q1 = sum(w^2); C1 = sum(m).
  AllReduce #1: per-feature partials [128, 4] of (S1, Q1, C1) summed over
    cores; then mu = S/C, var = Q/C - mu^2 (exact expansion of the masked
    batchnorm; count_nonzero == sum(mask) almost surely).
  pass 2 (pure SBUF, no HBM traffic): y = s*w + t*(w != 0) realized via
    ACT sigmoid/softplus with per-partition scale/bias + copy_predicated
    fixups at masked (w == 0) positions; gated = sig * softplus summed
    over the M=12 neighbors -> ns. Stats (S2, Q2, count) -> AllReduce #2.
  pass 3: out = softplus(atom + s2*ns + t2) masked by (ns != 0), TensorE-
    transposed back to natural layout and DMA'd out.

The only cross-core traffic is two [128, 4] fp32 AllReduces.
"""

import sys

sys.path.insert(0, "/opt/trn_rl_repo")

import numpy as np

import concourse.bass as bass
import concourse.bacc as bacc
import concourse.mybir as mybir
import concourse.tile as tile

F16 = mybir.dt.float16
F32 = mybir.dt.float32
I32 = mybir.dt.int32
I8 = mybir.dt.int8
AF = mybir.ActivationFunctionType
ALU = mybir.AluOpType

B, N, M, F, NF = 512, 64, 12, 64, 41
F2 = 2 * F          # 128
K2 = F2 + NF        # 169
EPS = 1e-5
LN2 = 0.6931471805599453
SIG0 = 0.5


def build(nc: bass.Bass, b_loc: int, n_cores: int, debug: bool = False, stage: int = 5):
    """Trace the per-core program into nc (SPMD: same program, all cores)."""
    RN = b_loc * N            # atom rows per core
    RR = b_loc * N * M        # edge rows per core
    assert RN % 128 == 0
    HALF = RR // 2            # stacked-half columns
    assert HALF % 12 == 0
    NOD2 = HALF // M          # nodes per stacked half
    CW = 768 if HALF % 768 == 0 else HALF     # pass-2 chunk width
    assert HALF % CW == 0 and CW % M == 0
    C2CONST = float(n_cores * RN * F)
    groups = [list(range(n_cores))]

    # ---- kernel I/O (per-core shards; flattened outer dims) ----
    atom_d = nc.dram_tensor("atom", [RN, F], F32, kind="ExternalInput").ap()
    nbr_d = nc.dram_tensor("nbr", [RR, NF], F32, kind="ExternalInput").ap()
    idx_d = nc.dram_tensor("idx", [RN, M], I32, kind="ExternalInput").ap()
    mask_d = nc.dram_tensor("mask", [RR, F2], I32, kind="ExternalInput").ap()
    w_d = nc.dram_tensor("gcw", [K2, F2], F32, kind="ExternalInput").ap()
    bias_d = nc.dram_tensor("gcb", [1, F2], F32, kind="ExternalInput").ap()
    g1_d = nc.dram_tensor("g1", [F2, 1], F32, kind="ExternalInput").ap()
    b1_d = nc.dram_tensor("b1", [F2, 1], F32, kind="ExternalInput").ap()
    g2_d = nc.dram_tensor("g2", [F, 1], F32, kind="ExternalInput").ap()
    b2_d = nc.dram_tensor("b2", [F, 1], F32, kind="ExternalInput").ap()
    out_d = nc.dram_tensor("out", [RN, F], F32, kind="ExternalOutput").ap()
    if debug:
        dbg_w = nc.dram_tensor("dbg_w", [F2, RR], F32, kind="ExternalOutput").ap()
        dbg_ns2 = nc.dram_tensor("dbg_ns2", [F2, NOD2], F32, kind="ExternalOutput").ap()
        dbg_ar1 = nc.dram_tensor("dbg_ar1", [F2, 4], F32, kind="ExternalOutput").ap()
        dbg_ar2 = nc.dram_tensor("dbg_ar2", [F2, 4], F32, kind="ExternalOutput").ap()
        dbg_st = nc.dram_tensor("dbg_st", [F2, 4], F32, kind="ExternalOutput").ap()

    # ---- collective bounce buffers ----
    cc1_in = nc.dram_tensor("cc1_in", [F2, 4], F32)
    cc1_out = nc.dram_tensor("cc1_out", [F2, 4], F32, addr_space="Shared")
    cc2_in = nc.dram_tensor("cc2_in", [F2, 4], F32)
    cc2_out = nc.dram_tensor("cc2_out", [F2, 4], F32, addr_space="Shared")

    with tile.TileContext(nc) as tc:
        with tc.tile_pool(name="res", bufs=1) as res:
            # ============ constants / weights ============
            iota_pp = res.tile([128, 128], I32)
            nc.gpsimd.iota(iota_pp[:, :], pattern=[[1, 128]], base=0,
                           channel_multiplier=-1)
            ident16 = res.tile([128, 128], F16)
            nc.vector.tensor_scalar(ident16[:, :], iota_pp[:, :], 0, None,
                                    ALU.is_equal)
            ident32 = res.tile([128, 128], F32)
            nc.vector.tensor_scalar(ident32[:, :], iota_pp[:, :], 0, None,
                                    ALU.is_equal)
            iota_n = res.tile([N, M * N], F16)
            nc.gpsimd.iota(iota_n[:, :], pattern=[[0, M * N]], base=0,
                           channel_multiplier=1,
                           allow_small_or_imprecise_dtypes=True)
            ones_col = res.tile([128, 1], F32)
            nc.vector.memset(ones_col[:, :], 1.0)
            ones_row = res.tile([1, 128], F32)
            nc.vector.memset(ones_row[:, :], 1.0)
            ones64 = res.tile([1, N], F16)
            nc.vector.memset(ones64[:, :], 1.0)

            wa = res.tile([128, F2], F16)
            nc.gpsimd.dma_start(out=wa[:, :], in_=w_d[0:128, :])
            wb = res.tile([NF, F2], F16)
            nc.gpsimd.dma_start(out=wb[:, :], in_=w_d[128:K2, :])
            bias_row = res.tile([1, F2], F16)
            nc.gpsimd.dma_start(out=bias_row[:, :], in_=bias_d[:, :])
            ones768 = res.tile([1, N * M], F16)
            nc.vector.memset(ones768[:, :], 1.0)
            g1c = res.tile([F2, 1], F32)
            nc.sync.dma_start(out=g1c[:, :], in_=g1_d[:, :])
            b1c = res.tile([F2, 1], F32)
            nc.sync.dma_start(out=b1c[:, :], in_=b1_d[:, :])
            g2c = res.tile([F, 1], F32)
            nc.sync.dma_start(out=g2c[:, :], in_=g2_d[:, :])
            b2c = res.tile([F, 1], F32)
            nc.sync.dma_start(out=b2c[:, :], in_=b2_d[:, :])

            # ============ atom layouts ============
            p1res_cm = tc.tile_pool(name="p1res", bufs=1)
            p1res = p1res_cm.__enter__()
            # n-major f16 copy: lhsT for the per-batch gather matmuls
            atom_nmaj = p1res.tile([N, b_loc, F], F16)
            nc.gpsimd.dma_start(
                out=atom_nmaj[:, :, :],
                in_=atom_d.rearrange("(b n) f -> n b f", n=N),
            )
            # row-major staging -> TensorE transpose -> atomT (f32 + f16)
            atomT32 = res.tile([F, RN], F32)
            atomT16 = p1res.tile([F, RN], F16)
            with (
                tc.tile_pool(name="astg", bufs=1) as astg,
                tc.tile_pool(name="aps", bufs=2, space="PSUM") as aps,
            ):
                anat = astg.tile([128, RN // 128, F], F32)
                nc.sync.dma_start(
                    out=anat[:, :, :],
                    in_=atom_d.rearrange("(a p) f -> p a f", p=128),
                )
                for j in range(RN // 128):
                    aT = aps.tile([F, 128], F32)
                    nc.tensor.transpose(aT[:, :], anat[:, j, :], ident32[:, :])
                    nc.scalar.copy(atomT32[:, j * 128:(j + 1) * 128], aT[:, :])
            nc.vector.tensor_copy(atomT16[:, :], atomT32[:, :])

            if stage == 1:
                stg = res.tile([F, 128], F32, tag="stg1")
                nc.vector.tensor_copy(stg[:, :], atomT32[:, 0:128])
                nc.sync.dma_start(out=out_d[0:F, 0:F], in_=stg[:, 0:F])
                p1res_cm.__exit__(None, None, None)
                return

            # ============ pass 1 ============
            w_all = res.tile([F2, RR], F16)       # masked preactivation cache
            # m_node predicate (exact 0/1), stacked-half layout
            mnode2 = res.tile([128, NOD2], F16)
            s1p = res.tile([F2, b_loc], F32)
            q1p = res.tile([F2, b_loc], F32)
            c1p = res.tile([F2, b_loc], F32)

            with (
                tc.tile_pool(name="mstg", bufs=2) as mstg,
                tc.tile_pool(name="nstg", bufs=2) as nstg,
                tc.tile_pool(name="istg", bufs=2) as istg,
                tc.tile_pool(name="ohst", bufs=2) as ohst,
                tc.tile_pool(name="ttab", bufs=2) as ttab,
                tc.tile_pool(name="mtsb", bufs=2) as mtsb,
                tc.tile_pool(name="qscr", bufs=2) as qscr,
                tc.tile_pool(name="ibc_ps", bufs=1, space="PSUM") as ibc_psp,
                tc.tile_pool(name="mt_ps", bufs=1, space="PSUM") as mt_psp,
                tc.tile_pool(name="nt_ps", bufs=1, space="PSUM") as nt_psp,
                tc.tile_pool(name="g_ps", bufs=1, space="PSUM") as g_psp,
                tc.tile_pool(name="z_ps", bufs=1, space="PSUM") as z_psp,
            ):
                for b in range(b_loc):
                    er = slice(b * N * M, (b + 1) * N * M)   # edge rows
                    # --- mask: cast-load, count, transpose ---
                    mnat = mstg.tile([128, 6, F2], F16)
                    nc.gpsimd.dma_start(
                        out=mnat[:, :, :],
                        in_=mask_d[er, :].rearrange("(a p) f -> p a f", p=128),
                    )
                    nc.vector.tensor_reduce(
                        c1p[:, b:b + 1],
                        mnat.rearrange("p a f -> p (a f)"),
                        axis=mybir.AxisListType.X, op=ALU.add)
                    mT = mt_psp.tile([F2, N * M], F16)
                    for j in range(6):
                        nc.tensor.transpose(mT[:, j * 128:(j + 1) * 128],
                                            mnat[:, j, :], ident16[:, :])
                    mt_sb = mtsb.tile([F2, N * M], F16)
                    nc.scalar.copy(mt_sb[:, :], mT[:, :])
                    mnsrc = mt_sb[0:F, :].rearrange(
                        "p (n m) -> p n m", m=M)[:, :, 0]
                    if b < b_loc // 2:
                        nc.vector.tensor_copy(
                            mnode2[0:F, b * N:(b + 1) * N], mnsrc)
                    else:
                        b2_ = b - b_loc // 2
                        nc.vector.tensor_copy(
                            mnode2[F:F2, b2_ * N:(b2_ + 1) * N], mnsrc)
                    # --- one-hot from neighbor indices ---
                    idx16 = istg.tile([1, N * M], F16)
                    nc.gpsimd.dma_start(
                        out=idx16[:, :],
                        in_=idx_d[b * N:(b + 1) * N, :].rearrange(
                            "(one n) m -> one (n m)", one=1),
                    )
                    ibc_ps = ibc_psp.tile([N, N * M], F32)
                    for lo, hi in ((0, 512), (512, N * M)):
                        nc.tensor.matmul(ibc_ps[:, lo:hi], ones64[:, :],
                                         idx16[:, lo:hi], start=True,
                                         stop=True)
                    onehot = ohst.tile([N, N * M], F16, tag="onehot")
                    nc.vector.tensor_tensor(onehot[:, :], iota_n[:, :],
                                            ibc_ps[:, :], ALU.is_equal)
                    # --- gather matmul + moving operand assembly ---
                    g_ps = g_psp.tile([F, N * M], F32)
                    for lo, hi in ((0, 512), (512, N * M)):
                        nc.tensor.matmul(g_ps[:, lo:hi], atom_nmaj[:, b, :],
                                         onehot[:, lo:hi], start=True,
                                         stop=True)
                    tta = ttab.tile([F2, N * M], F16, tag="tta")
                    nc.scalar.copy(tta[F:F2, :], g_ps[:, :])
                    nc.scalar.copy(
                        tta[0:F, :].rearrange("p (n m) -> p n m", m=M),
                        atomT16[:, b * N:(b + 1) * N].rearrange(
                            "p (n one) -> p n one", one=1
                        ).broadcast_to([F, N, M]),
                    )
                    # --- nbr: cast-load, transpose ---
                    nnat = nstg.tile([128, 6, NF], F16)
                    nc.gpsimd.dma_start(
                        out=nnat[:, :, :],
                        in_=nbr_d[er, :].rearrange("(a p) f -> p a f", p=128),
                    )
                    nT = nt_psp.tile([NF, N * M], F16)
                    for j in range(6):
                        nc.tensor.transpose(nT[:, j * 128:(j + 1) * 128],
                                            nnat[:, j, :], ident16[:, :])
                    ttb = ttab.tile([NF, N * M], F16, tag="ttb")
                    nc.scalar.copy(ttb[:, :], nT[:, :])
                    # --- z matmuls (accumulate over both K groups) ---
                    z_ps = z_psp.tile([F2, N * M], F32)
                    for lo, hi in ((0, 512), (512, N * M)):
                        nc.tensor.matmul(z_ps[:, lo:hi], wa[:, :],
                                         tta[:, lo:hi], start=True,
                                         stop=False)
                        nc.tensor.matmul(z_ps[:, lo:hi], wb[:, :],
                                         ttb[:, lo:hi], start=False,
                                         stop=False)
                        nc.tensor.matmul(z_ps[:, lo:hi], bias_row[:, :],
                                         ones768[:, lo:hi], start=False,
                                         stop=True)
                    # --- w = z*m with fused S1 partial; Q1 on ACT ---
                    nc.vector.tensor_tensor(w_all[:, er], z_ps[:, :],
                                            mt_sb[:, :], ALU.mult)
                    nc.vector.tensor_reduce(s1p[:, b:b + 1], w_all[:, er],
                                            axis=mybir.AxisListType.X,
                                            op=ALU.add)
                    qs = qscr.tile([F2, N * M], F16)
                    nc.scalar.activation(qs[:, :], w_all[:, er], AF.Square,
                                         accum_out=q1p[:, b:b + 1])

            p1res_cm.__exit__(None, None, None)

            if stage == 2:
                stg2 = res.tile([F2, 64], F32, tag="stg2")
                nc.scalar.copy(stg2[:, :], w_all[:, 0:64])
                nc.sync.dma_start(out=out_d[0:F2, 0:F],
                                  in_=stg2[:, 0:F])
                return

            # ============ AllReduce #1 + scalar chain ============
            ar1 = res.tile([F2, 4], F32)
            nc.vector.tensor_reduce(ar1[:, 0:1], s1p[:, :],
                                    axis=mybir.AxisListType.X, op=ALU.add)
            nc.vector.tensor_reduce(ar1[:, 1:2], q1p[:, :],
                                    axis=mybir.AxisListType.X, op=ALU.add)
            nc.vector.tensor_reduce(ar1[:, 2:3], c1p[:, :],
                                    axis=mybir.AxisListType.X, op=ALU.add)
            nc.vector.memset(ar1[:, 3:4], 0.0)
            nc.sync.dma_start(out=cc1_in.ap(), in_=ar1[:, :])
            if n_cores > 1:
                nc.gpsimd.collective_compute(
                    "AllReduce", ALU.add, replica_groups=groups,
                    ins=[cc1_in.ap().opt()], outs=[cc1_out.ap().opt()])
            else:
                nc.sync.dma_start(out=cc1_out.ap(), in_=cc1_in.ap())
            arb1 = res.tile([F2, 4], F32)
            nc.sync.dma_start(out=arb1[:, :], in_=cc1_out.ap())

            def stats_chain(arb, extra_count, ps_sm):
                """arb [128,4] cols (S, Q, negC_or_C, -) -> bcast [128,2] =
                (istd, mu*istd) columns."""
                sums_ps = ps_sm.tile([1, 4], F32, tag="sums")
                nc.tensor.matmul(sums_ps[:, :], ones_col[:, :], arb[:, :],
                                 start=True, stop=True)
                sc = res.tile([1, 12], F32, tag=f"sc{id(arb)}")
                nc.scalar.copy(sc[:, 0:4], sums_ps[:, :])
                # cols: 0=S 1=Q 2=C(+extra) 3=- 4=Cinv 5=mu 6=Q/C 7=mu^2
                #       8=var(+eps) 9=std 10=istd 11=mu*istd
                if extra_count:
                    nc.vector.tensor_scalar(sc[:, 2:3], sc[:, 2:3],
                                            float(extra_count), None, ALU.add)
                nc.vector.reciprocal(sc[:, 4:5], sc[:, 2:3])
                nc.vector.tensor_tensor(sc[:, 5:6], sc[:, 0:1], sc[:, 4:5],
                                        ALU.mult)
                nc.vector.tensor_tensor(sc[:, 6:7], sc[:, 1:2], sc[:, 4:5],
                                        ALU.mult)
                nc.vector.tensor_tensor(sc[:, 7:8], sc[:, 5:6], sc[:, 5:6],
                                        ALU.mult)
                nc.vector.tensor_tensor(sc[:, 8:9], sc[:, 6:7], sc[:, 7:8],
                                        ALU.subtract)
                nc.vector.tensor_scalar(sc[:, 8:9], sc[:, 8:9], EPS, None,
                                        ALU.add)
                nc.scalar.activation(sc[:, 9:10], sc[:, 8:9], AF.Sqrt)
                nc.vector.reciprocal(sc[:, 10:11], sc[:, 9:10])
                nc.vector.tensor_tensor(sc[:, 11:12], sc[:, 5:6],
                                        sc[:, 10:11], ALU.mult)
                bc_ps = ps_sm.tile([128, 2], F32, tag="bc")
                nc.tensor.matmul(bc_ps[:, :], ones_row[:, :], sc[:, 10:12],
                                 start=True, stop=True)
                bc = res.tile([128, 2], F32, tag=f"bc{id(arb)}")
                nc.scalar.copy(bc[:, :], bc_ps[:, :])
                return bc

            with tc.tile_pool(name="ps_sm1", bufs=1, space="PSUM") as ps_sm:
                bc1 = stats_chain(arb1, 0, ps_sm)
            s1c = res.tile([F2, 1], F32)      # istd * gamma
            t1c = res.tile([F2, 1], F32)      # beta - mu * istd * gamma
            nc.vector.tensor_tensor(s1c[:, :], g1c[:, :], bc1[:, 0:1],
                                    ALU.mult)
            nc.vector.tensor_tensor(t1c[:, :], g1c[:, :], bc1[:, 1:2],
                                    ALU.mult)
            nc.vector.tensor_sub(t1c[:, :], b1c[:, :], t1c[:, :])
            # duplicated (stacked-half) forms
            sf = res.tile([128, 1], F32)
            tf = res.tile([128, 1], F32)
            sg = res.tile([128, 1], F32)
            tg = res.tile([128, 1], F32)
            for dst, src in ((sf, s1c), (tf, t1c)):
                nc.vector.tensor_copy(dst[0:64, :], src[0:64, :])
                nc.vector.tensor_copy(dst[64:128, :], src[0:64, :])
            for dst, src in ((sg, s1c), (tg, t1c)):
                nc.vector.tensor_copy(dst[0:64, :], src[64:128, :])
                nc.vector.tensor_copy(dst[64:128, :], src[64:128, :])

            if stage == 3:
                nc.sync.dma_start(out=out_d[0:F2, 0:1], in_=s1c[:, :])
                return

            # ============ pass 2 ============
            ns2 = res.tile([128, NOD2], F32)
            with tc.tile_pool(name="p2", bufs=2) as p2:
                for c in range(HALF // CW):
                    s0 = slice(c * CW, (c + 1) * CW)
                    s1 = slice(HALF + c * CW, HALF + (c + 1) * CW)
                    wf = p2.tile([128, CW], F16, tag="wf")
                    nc.vector.tensor_copy(wf[0:64, :], w_all[0:F, s0])
                    nc.vector.tensor_copy(wf[64:128, :], w_all[0:F, s1])
                    wc = p2.tile([128, CW], F16, tag="wc")
                    nc.vector.tensor_copy(wc[0:64, :], w_all[F:F2, s0])
                    nc.vector.tensor_copy(wc[64:128, :], w_all[F:F2, s1])
                    gf = p2.tile([128, CW], F16, tag="gf")
                    nc.vector.memset(gf[:, :], SIG0)
                    sig = p2.tile([128, CW], F16, tag="sig")
                    nc.scalar.activation(sig[:, :], wf[:, :], AF.Sigmoid,
                                         bias=tf[:, :], scale=sf[:, :])
                    pif = p2.tile([128, CW], I8, tag="pif")
                    nc.vector.tensor_scalar(pif[:, :], wf[:, :], 0.0, None,
                                            ALU.not_equal)
                    nc.vector.copy_predicated(gf[:, :], pif[:, :], sig[:, :])
                    gc_ = p2.tile([128, CW], F16, tag="gc")
                    nc.vector.memset(gc_[:, :], LN2)
                    ex = p2.tile([128, CW], F32, tag="ex")
                    nc.scalar.activation(ex[:, :], wc[:, :], AF.Exp,
                                         bias=tg[:, :], scale=sg[:, :])
                    sof = p2.tile([128, CW], F16, tag="sof")
                    nc.scalar.activation(sof[:, :], ex[:, :], AF.Ln, bias=1.0)
                    pic = p2.tile([128, CW], I8, tag="pic")
                    nc.vector.tensor_scalar(pic[:, :], wc[:, :], 0.0, None,
                                            ALU.not_equal)
                    nc.vector.copy_predicated(gc_[:, :], pic[:, :], sof[:, :])
                    gtd = p2.tile([128, CW], F16, tag="gtd")
                    nc.vector.tensor_mul(gtd[:, :], gf[:, :], gc_[:, :])
                    nsr = p2.tile([128, CW // M], F32, tag="nsr")
                    nc.vector.tensor_reduce(
                        nsr[:, :],
                        gtd.rearrange("p (n m) -> p n m", m=M),
                        axis=mybir.AxisListType.X, op=ALU.add)
                    nsl = slice(c * (CW // M), (c + 1) * (CW // M))
                    pim = p2.tile([128, CW // M], I8, tag="pim")
                    nc.vector.tensor_scalar(pim[:, :], mnode2[:, nsl], 0.0,
                                            None, ALU.not_equal)
                    nc.vector.memset(ns2[:, nsl], 0.0)
                    nc.vector.copy_predicated(ns2[:, nsl], pim[:, :],
                                              nsr[:, :])

            if debug:
                with tc.tile_pool(name="dbgw", bufs=1) as dbgp:
                    for j in range(RR // 512):
                        dw = dbgp.tile([F2, 512], F32, tag="dw")
                        nc.vector.tensor_copy(dw[:, :], w_all[:, j * 512:(j + 1) * 512])
                        nc.sync.dma_start(out=dbg_w[:, j * 512:(j + 1) * 512], in_=dw[:, :])
                    nc.sync.dma_start(out=dbg_ar1, in_=arb1[:, :])
                    dst = dbgp.tile([F2, 4], F32, tag="dst")
                    nc.vector.tensor_copy(dst[:, 0:1], s1c[:, :])
                    nc.vector.tensor_copy(dst[:, 1:2], t1c[:, :])
                    nc.vector.tensor_copy(dst[:, 2:3], sf[:, :])
                    nc.vector.tensor_copy(dst[:, 3:4], tf[:, :])
                    nc.sync.dma_start(out=dbg_st, in_=dst[:, :])

            if stage == 4:
                nc.sync.dma_start(out=out_d[0:F2, 0:F], in_=ns2[:, 0:F])
                return

            # ============ stats 2 + AllReduce #2 ============
            ar2 = res.tile([F2, 4], F32)
            nc.vector.tensor_reduce(ar2[:, 0:1], ns2[:, :],
                                    axis=mybir.AxisListType.X, op=ALU.add)
            with tc.tile_pool(name="q2s", bufs=1) as q2s:
                q2scr = q2s.tile([128, NOD2], F16)
                nc.scalar.activation(q2scr[:, :], ns2[:, :], AF.Square,
                                     accum_out=ar2[:, 1:2])
                eqt = q2s.tile([128, NOD2], F16)
                nc.vector.tensor_scalar(eqt[:, :], ns2[:, :], 0.0, None,
                                        ALU.is_equal)
                nc.vector.tensor_reduce(ar2[:, 2:3], eqt[:, :],
                                        axis=mybir.AxisListType.X,
                                        op=ALU.add, negate=True)
            nc.vector.memset(ar2[:, 3:4], 0.0)
            nc.sync.dma_start(out=cc2_in.ap(), in_=ar2[:, :])
            if n_cores > 1:
                nc.gpsimd.collective_compute(
                    "AllReduce", ALU.add, replica_groups=groups,
                    ins=[cc2_in.ap().opt()], outs=[cc2_out.ap().opt()])
            else:
                nc.sync.dma_start(out=cc2_out.ap(), in_=cc2_in.ap())
            arb2 = res.tile([F2, 4], F32)
            nc.sync.dma_start(out=arb2[:, :], in_=cc2_out.ap())
            with tc.tile_pool(name="ps_sm2", bufs=1, space="PSUM") as ps_sm:
                bc2 = stats_chain(arb2, C2CONST, ps_sm)
            if debug:
                nc.sync.dma_start(out=dbg_ar2, in_=arb2[:, :])
                nc.sync.dma_start(out=dbg_ns2, in_=ns2[:, :])
            s2c = res.tile([F, 1], F32)
            t2c = res.tile([F, 1], F32)
            nc.vector.tensor_tensor(s2c[:, :], g2c[:, :], bc2[0:F, 0:1],
                                    ALU.mult)
            nc.vector.tensor_tensor(t2c[:, :], g2c[:, :], bc2[0:F, 1:2],
                                    ALU.mult)
            nc.vector.tensor_sub(t2c[:, :], b2c[:, :], t2c[:, :])

            # ============ pass 3 ============
            CP = 512 if (RN // 2) % 512 == 0 else RN // 2
            TS = min(128, CP)
            with (
                tc.tile_pool(name="p3", bufs=2) as p3,
                tc.tile_pool(name="ops", bufs=2, space="PSUM") as ops,
            ):
                for c in range(RN // CP):
                    cs = slice(c * CP, (c + 1) * CP)
                    half, col = divmod(c * CP, RN // 2)
                    nsl = slice(half * 64, half * 64 + 64)
                    ns_c = p3.tile([F, CP], F32, tag="ns")
                    nc.vector.tensor_copy(ns_c[:, :],
                                          ns2[nsl, col:col + CP])
                    y2 = p3.tile([F, CP], F32, tag="y2")
                    nc.vector.tensor_scalar(y2[:, :], ns_c[:, :], s2c[:, :],
                                            t2c[:, :], ALU.mult, ALU.add)
                    nc.vector.tensor_add(y2[:, :], y2[:, :], atomT32[:, cs])
                    ex3 = p3.tile([F, CP], F32, tag="ex3")
                    nc.scalar.activation(ex3[:, :], y2[:, :], AF.Exp)
                    sp = p3.tile([F, CP], F32, tag="sp")
                    nc.scalar.activation(sp[:, :], ex3[:, :], AF.Ln, bias=1.0)
                    fin = p3.tile([F, CP], F32, tag="fin")
                    nc.vector.memset(fin[:, :], 0.0)
                    pin = p3.tile([F, CP], I8, tag="pin")
                    nc.vector.tensor_scalar(pin[:, :], ns_c[:, :], 0.0, None,
                                            ALU.not_equal)
                    nc.vector.copy_predicated(fin[:, :], pin[:, :], sp[:, :])
                    for j in range(CP // TS):
                        op = ops.tile([TS, F], F32)
                        nc.tensor.transpose(op[:, :],
                                            fin[:, j * TS:(j + 1) * TS],
                                            ident32[0:F, 0:F])
                        ob = p3.tile([TS, F], F32, tag="ob")
                        nc.scalar.copy(ob[:, :], op[:, :])
                        r0 = c * CP + j * TS
                        nc.sync.dma_start(out=out_d[r0:r0 + TS, :],
                                          in_=ob[:, :])


def make_in_maps(inputs, b_loc, n_cores):
    """Shard full inputs over cores (batch-split); replicate weights."""
    atom = np.ascontiguousarray(inputs["atom_fea"], dtype=np.float32)
    nbr = np.ascontiguousarray(inputs["nbr_fea"], dtype=np.float32)
    idx = np.ascontiguousarray(inputs["nbr_fea_idx"], dtype=np.int32)
    mask = np.ascontiguousarray(inputs["mask"], dtype=np.int32)
    w = np.ascontiguousarray(inputs["gc_W"], dtype=np.float32)
    bias = np.ascontiguousarray(inputs["gc_bias"], dtype=np.float32)
    g1 = np.ascontiguousarray(inputs["gamma_1"], dtype=np.float32)
    b1 = np.ascontiguousarray(inputs["beta_1"], dtype=np.float32)
    g2 = np.ascontiguousarray(inputs["gamma_2"], dtype=np.float32)
    b2 = np.ascontiguousarray(inputs["beta_2"], dtype=np.float32)
    in_maps = []
    for i in range(n_cores):
        bs = slice(i * b_loc, (i + 1) * b_loc)
        in_maps.append({
            "atom": atom[bs].reshape(b_loc * N, F),
            "nbr": nbr[bs].reshape(b_loc * N * M, NF),
            "idx": idx[bs].reshape(b_loc * N, M),
            "mask": mask[bs].reshape(b_loc * N * M, F2),
            "gcw": w,
            "gcb": bias.reshape(1, F2),
            "g1": g1.reshape(F2, 1),
            "b1": b1.reshape(F2, 1),
            "g2": g2.reshape(F, 1),
            "b2": b2.reshape(F, 1),
        })
    return in_maps


_CACHED = {}


def _get_nc(b_loc, n_cores):
    key = (b_loc, n_cores)
    if key not in _CACHED:
        nc = bacc.Bacc("TRN2", target_bir_lowering=False, debug=False,
                       num_devices=n_cores)
        build(nc, b_loc, n_cores)
        nc.compile()
        _CACHED[key] = nc
    return _CACHED[key]


LAST_RESULTS = None


def kernel(**inputs) -> np.ndarray:
    import os
    from concourse.bass_utils import run_bass_kernel_spmd

    global LAST_RESULTS
    n_cores = 8
    b_loc = B // n_cores
    nc = _get_nc(b_loc, n_cores)
    in_maps = make_in_maps(inputs, b_loc, n_cores)
    trace = os.environ.get("CGC_TRACE") == "1"
    res = run_bass_kernel_spmd(nc, in_maps, core_ids=list(range(n_cores)),
                               trace=trace)
    LAST_RESULTS = res
    out = np.concatenate(
        [res.results[i]["out"].reshape(b_loc, N, F) for i in range(n_cores)],
        axis=0)
    return out.astype(np.float32, copy=False)


# revision 23
# speedup vs baseline: 1.0418x; 1.0418x over previous
"""CrystalGraphConv (CGCNN conv layer) Trainium2 kernel, 8-core data-parallel.

Strategy
--------
Data-parallel over batch B (512 -> 64 per core). Per core, everything is
computed in a feature-major ("transposed") SBUF layout so the gated-conv
matmul z = [atom_exp | gathered | nbr] @ W runs with W as the stationary
operand and the concatenated features streaming as the moving operand:

  pass 1 (streams all inputs from HBM exactly once):
    - mask int32 -> f16 via cast-DMA, transposed on TensorE (via identity
      matmul) to m^T[128 feat, rows]
    - neighbor gather done as a one-hot matmul per batch:
        gathered^T = atom_b.T @ onehot^T, onehot^T[n, r] = (idx[r] == n)
    - atom_exp term uses a free-axis broadcast AP (each column x12)
    - z^T accumulated in PSUM over two K-groups (W rows 0:128, 128:169 +
      a ones-row carrying gc_bias)
    - w = z * m  (masked preactivation) cached in SBUF as f16 [128, rows];
      fused reduce accumulates S1 = sum(w); ACT engine accumulates
      Q1 = sum(w^2); C1 = sum(m).
  AllReduce #1: per-feature partials [128, 4] of (S1, Q1, C1) summed over
    cores; then mu = S/C, var = Q/C - mu^2 (exact expansion of the masked
    batchnorm; count_nonzero == sum(mask) almost surely).
  pass 2 (pure SBUF, no HBM traffic): y = s*w + t*(w != 0) realized via
    ACT sigmoid/softplus with per-partition scale/bias + copy_predicated
    fixups at masked (w == 0) positions; gated = sig * softplus summed
    over the M=12 neighbors -> ns. Stats (S2, Q2, count) -> AllReduce #2.
  pass 3: out = softplus(atom + s2*ns + t2) masked by (ns != 0), TensorE-
    transposed back to natural layout and DMA'd out.

The only cross-core traffic is two [128, 4] fp32 AllReduces.
"""

import sys

sys.path.insert(0, "/opt/trn_rl_repo")

import numpy as np

import concourse.bass as bass
import concourse.bacc as bacc
import concourse.mybir as mybir
import concourse.tile as tile

F16 = mybir.dt.float16
F32 = mybir.dt.float32
I32 = mybir.dt.int32
I8 = mybir.dt.int8
AF = mybir.ActivationFunctionType
ALU = mybir.AluOpType

B, N, M, F, NF = 512, 64, 12, 64, 41
F2 = 2 * F          # 128
K2 = F2 + NF        # 169
EPS = 1e-5
LN2 = 0.6931471805599453
SIG0 = 0.5


def build(nc: bass.Bass, b_loc: int, n_cores: int, debug: bool = False, stage: int = 5):
    """Trace the per-core program into nc (SPMD: same program, all cores)."""
    RN = b_loc * N            # atom rows per core
    RR = b_loc * N * M        # edge rows per core
    assert RN % 128 == 0
    HALF = RR // 2            # stacked-half columns
    assert HALF % 12 == 0
    NOD2 = HALF // M          # nodes per stacked half
    CW = 768 if HALF % 768 == 0 else HALF     # pass-2 chunk width
    assert HALF % CW == 0 and CW % M == 0
    C2CONST = float(n_cores * RN * F)
    groups = [list(range(n_cores))]

    # ---- kernel I/O (per-core shards; flattened outer dims) ----
    atom_d = nc.dram_tensor("atom", [RN, F], F32, kind="ExternalInput").ap()
    nbr_d = nc.dram_tensor("nbr", [RR, NF], F32, kind="ExternalInput").ap()
    idx_d = nc.dram_tensor("idx", [RN, M], I32, kind="ExternalInput").ap()
    mask_d = nc.dram_tensor("mask", [RR, F2], I32, kind="ExternalInput").ap()
    w_d = nc.dram_tensor("gcw", [K2, F2], F32, kind="ExternalInput").ap()
    bias_d = nc.dram_tensor("gcb", [1, F2], F32, kind="ExternalInput").ap()
    g1_d = nc.dram_tensor("g1", [F2, 1], F32, kind="ExternalInput").ap()
    b1_d = nc.dram_tensor("b1", [F2, 1], F32, kind="ExternalInput").ap()
    g2_d = nc.dram_tensor("g2", [F, 1], F32, kind="ExternalInput").ap()
    b2_d = nc.dram_tensor("b2", [F, 1], F32, kind="ExternalInput").ap()
    out_d = nc.dram_tensor("out", [RN, F], F32, kind="ExternalOutput").ap()
    if debug:
        dbg_w = nc.dram_tensor("dbg_w", [F2, RR], F32, kind="ExternalOutput").ap()
        dbg_ns2 = nc.dram_tensor("dbg_ns2", [F2, NOD2], F32, kind="ExternalOutput").ap()
        dbg_ar1 = nc.dram_tensor("dbg_ar1", [F2, 4], F32, kind="ExternalOutput").ap()
        dbg_ar2 = nc.dram_tensor("dbg_ar2", [F2, 4], F32, kind="ExternalOutput").ap()
        dbg_st = nc.dram_tensor("dbg_st", [F2, 4], F32, kind="ExternalOutput").ap()

    # ---- collective bounce buffers ----
    cc1_in = nc.dram_tensor("cc1_in", [F2, 4], F32)
    cc1_out = nc.dram_tensor("cc1_out", [F2, 4], F32, addr_space="Shared")
    cc2_in = nc.dram_tensor("cc2_in", [F2, 4], F32)
    cc2_out = nc.dram_tensor("cc2_out", [F2, 4], F32, addr_space="Shared")

    with tile.TileContext(nc) as tc:
        with tc.tile_pool(name="res", bufs=1) as res:
            # ============ constants / weights ============
            iota_pp = res.tile([128, 128], I32)
            nc.gpsimd.iota(iota_pp[:, :], pattern=[[1, 128]], base=0,
                           channel_multiplier=-1)
            ident16 = res.tile([128, 128], F16)
            nc.vector.tensor_scalar(ident16[:, :], iota_pp[:, :], 0, None,
                                    ALU.is_equal)
            ident32 = res.tile([128, 128], F32)
            nc.vector.tensor_scalar(ident32[:, :], iota_pp[:, :], 0, None,
                                    ALU.is_equal)
            iota_n = res.tile([N, M * N], F16)
            nc.gpsimd.iota(iota_n[:, :], pattern=[[0, M * N]], base=0,
                           channel_multiplier=1,
                           allow_small_or_imprecise_dtypes=True)
            ones_col = res.tile([128, 1], F32)
            nc.vector.memset(ones_col[:, :], 1.0)
            ones_row = res.tile([1, 128], F32)
            nc.vector.memset(ones_row[:, :], 1.0)
            ones64 = res.tile([1, N], F16)
            nc.vector.memset(ones64[:, :], 1.0)

            wa = res.tile([128, F2], F16)
            nc.gpsimd.dma_start(out=wa[:, :], in_=w_d[0:128, :])
            wb = res.tile([NF, F2], F16)
            nc.gpsimd.dma_start(out=wb[:, :], in_=w_d[128:K2, :])
            bias_row = res.tile([1, F2], F16)
            nc.gpsimd.dma_start(out=bias_row[:, :], in_=bias_d[:, :])
            ones768 = res.tile([1, N * M], F16)
            nc.vector.memset(ones768[:, :], 1.0)
            g1c = res.tile([F2, 1], F32)
            nc.sync.dma_start(out=g1c[:, :], in_=g1_d[:, :])
            b1c = res.tile([F2, 1], F32)
            nc.sync.dma_start(out=b1c[:, :], in_=b1_d[:, :])
            g2c = res.tile([F, 1], F32)
            nc.sync.dma_start(out=g2c[:, :], in_=g2_d[:, :])
            b2c = res.tile([F, 1], F32)
            nc.sync.dma_start(out=b2c[:, :], in_=b2_d[:, :])

            # ============ atom layouts ============
            p1res_cm = tc.tile_pool(name="p1res", bufs=1)
            p1res = p1res_cm.__enter__()
            # n-major f16 copy: lhsT for the per-batch gather matmuls
            atom_nmaj = p1res.tile([N, b_loc, F], F16)
            nc.gpsimd.dma_start(
                out=atom_nmaj[:, :, :],
                in_=atom_d.rearrange("(b n) f -> n b f", n=N),
            )
            # row-major staging -> TensorE transpose -> atomT (f32 + f16)
            atomT32 = res.tile([F, RN], F32)
            atomT16 = p1res.tile([F, RN], F16)
            with (
                tc.tile_pool(name="astg", bufs=1) as astg,
                tc.tile_pool(name="aps", bufs=2, space="PSUM") as aps,
            ):
                anat = astg.tile([128, RN // 128, F], F32)
                nc.sync.dma_start(
                    out=anat[:, :, :],
                    in_=atom_d.rearrange("(a p) f -> p a f", p=128),
                )
                for j in range(RN // 128):
                    aT = aps.tile([F, 128], F32)
                    nc.tensor.transpose(aT[:, :], anat[:, j, :], ident32[:, :])
                    nc.scalar.copy(atomT32[:, j * 128:(j + 1) * 128], aT[:, :])
            nc.vector.tensor_copy(atomT16[:, :], atomT32[:, :])

            if stage == 1:
                stg = res.tile([F, 128], F32, tag="stg1")
                nc.vector.tensor_copy(stg[:, :], atomT32[:, 0:128])
                nc.sync.dma_start(out=out_d[0:F, 0:F], in_=stg[:, 0:F])
                p1res_cm.__exit__(None, None, None)
                return

            # ============ pass 1 ============
            w_all = res.tile([F2, RR], F16)       # masked preactivation cache
            # m_node predicate (exact 0/1), stacked-half layout
            mnode2 = res.tile([128, NOD2], F16)
            s1p = res.tile([F2, b_loc], F32)
            q1p = res.tile([F2, b_loc], F32)
            c1p = res.tile([F2, b_loc], F32)

            with (
                tc.tile_pool(name="mstg", bufs=3) as mstg,
                tc.tile_pool(name="nstg", bufs=3) as nstg,
                tc.tile_pool(name="istg", bufs=3) as istg,
                tc.tile_pool(name="ohst", bufs=3) as ohst,
                tc.tile_pool(name="ttab", bufs=3) as ttab,
                tc.tile_pool(name="mtsb", bufs=3) as mtsb,
                tc.tile_pool(name="qscr", bufs=3) as qscr,
                tc.tile_pool(name="ibc_ps", bufs=1, space="PSUM") as ibc_psp,
                tc.tile_pool(name="mt_ps", bufs=1, space="PSUM") as mt_psp,
                tc.tile_pool(name="nt_ps", bufs=1, space="PSUM") as nt_psp,
                tc.tile_pool(name="g_ps", bufs=1, space="PSUM") as g_psp,
                tc.tile_pool(name="z_ps", bufs=2, space="PSUM") as z_psp,
            ):
                for b in range(b_loc):
                    er = slice(b * N * M, (b + 1) * N * M)   # edge rows
                    # --- mask: cast-load, count, transpose ---
                    mnat = mstg.tile([128, 6, F2], F16)
                    nc.gpsimd.dma_start(
                        out=mnat[:, :, :],
                        in_=mask_d[er, :].rearrange("(a p) f -> p a f", p=128),
                    )
                    nc.vector.tensor_reduce(
                        c1p[:, b:b + 1],
                        mnat.rearrange("p a f -> p (a f)"),
                        axis=mybir.AxisListType.X, op=ALU.add)
                    mT = mt_psp.tile([F2, N * M], F16)
                    for j in range(6):
                        nc.tensor.transpose(mT[:, j * 128:(j + 1) * 128],
                                            mnat[:, j, :], ident16[:, :])
                    mt_sb = mtsb.tile([F2, N * M], F16)
                    nc.scalar.copy(mt_sb[:, :], mT[:, :])
                    mnsrc = mt_sb[0:F, :].rearrange(
                        "p (n m) -> p n m", m=M)[:, :, 0]
                    if b < b_loc // 2:
                        nc.vector.tensor_copy(
                            mnode2[0:F, b * N:(b + 1) * N], mnsrc)
                    else:
                        b2_ = b - b_loc // 2
                        nc.vector.tensor_copy(
                            mnode2[F:F2, b2_ * N:(b2_ + 1) * N], mnsrc)
                    # --- one-hot from neighbor indices ---
                    idx16 = istg.tile([1, N * M], F16)
                    nc.gpsimd.dma_start(
                        out=idx16[:, :],
                        in_=idx_d[b * N:(b + 1) * N, :].rearrange(
                            "(one n) m -> one (n m)", one=1),
                    )
                    onehot = ohst.tile([N, N * M], F16, tag="onehot")
                    for lo, hi in ((0, 384), (384, N * M)):
                        ibc_ps = ibc_psp.tile([N, 384], F32, tag="ibc")
                        nc.tensor.matmul(ibc_ps[:, :], ones64[:, :],
                                         idx16[:, lo:hi], start=True,
                                         stop=True)
                        nc.vector.tensor_tensor(onehot[:, lo:hi],
                                                iota_n[:, lo:hi],
                                                ibc_ps[:, :], ALU.is_equal)
                    # --- gather matmul + moving operand assembly ---
                    tta = ttab.tile([F2, N * M], F16, tag="tta")
                    for lo, hi in ((0, 384), (384, N * M)):
                        g_ps = g_psp.tile([F, 384], F32, tag="g")
                        nc.tensor.matmul(g_ps[:, :], atom_nmaj[:, b, :],
                                         onehot[:, lo:hi], start=True,
                                         stop=True)
                        nc.scalar.copy(tta[F:F2, lo:hi], g_ps[:, :])
                    nc.scalar.copy(
                        tta[0:F, :].rearrange("p (n m) -> p n m", m=M),
                        atomT16[:, b * N:(b + 1) * N].rearrange(
                            "p (n one) -> p n one", one=1
                        ).broadcast_to([F, N, M]),
                    )
                    # --- nbr: cast-load, transpose ---
                    nnat = nstg.tile([128, 6, NF], F16)
                    nc.gpsimd.dma_start(
                        out=nnat[:, :, :],
                        in_=nbr_d[er, :].rearrange("(a p) f -> p a f", p=128),
                    )
                    nT = nt_psp.tile([NF, N * M], F16)
                    for j in range(6):
                        nc.tensor.transpose(nT[:, j * 128:(j + 1) * 128],
                                            nnat[:, j, :], ident16[:, :])
                    ttb = ttab.tile([NF, N * M], F16, tag="ttb")
                    nc.scalar.copy(ttb[:, :], nT[:, :])
                    # --- z matmuls (accumulate over both K groups) ---
                    z_ps = z_psp.tile([F2, N * M], F32)
                    for lo, hi in ((0, 512), (512, N * M)):
                        nc.tensor.matmul(z_ps[:, lo:hi], wa[:, :],
                                         tta[:, lo:hi], start=True,
                                         stop=False)
                        nc.tensor.matmul(z_ps[:, lo:hi], wb[:, :],
                                         ttb[:, lo:hi], start=False,
                                         stop=False)
                        nc.tensor.matmul(z_ps[:, lo:hi], bias_row[:, :],
                                         ones768[:, lo:hi], start=False,
                                         stop=True)
                    # --- w = z*m with fused S1 partial; Q1 on ACT ---
                    nc.vector.tensor_tensor(w_all[:, er], z_ps[:, :],
                                            mt_sb[:, :], ALU.mult)
                    nc.vector.tensor_reduce(s1p[:, b:b + 1], w_all[:, er],
                                            axis=mybir.AxisListType.X,
                                            op=ALU.add)
                    qs = qscr.tile([F2, N * M], F16)
                    nc.scalar.activation(qs[:, :], w_all[:, er], AF.Square,
                                         accum_out=q1p[:, b:b + 1])

            p1res_cm.__exit__(None, None, None)

            if stage == 2:
                stg2 = res.tile([F2, 64], F32, tag="stg2")
                nc.scalar.copy(stg2[:, :], w_all[:, 0:64])
                nc.sync.dma_start(out=out_d[0:F2, 0:F],
                                  in_=stg2[:, 0:F])
                return

            # ============ AllReduce #1 + scalar chain ============
            ar1 = res.tile([F2, 4], F32)
            nc.vector.tensor_reduce(ar1[:, 0:1], s1p[:, :],
                                    axis=mybir.AxisListType.X, op=ALU.add)
            nc.vector.tensor_reduce(ar1[:, 1:2], q1p[:, :],
                                    axis=mybir.AxisListType.X, op=ALU.add)
            nc.vector.tensor_reduce(ar1[:, 2:3], c1p[:, :],
                                    axis=mybir.AxisListType.X, op=ALU.add)
            nc.vector.memset(ar1[:, 3:4], 0.0)
            nc.sync.dma_start(out=cc1_in.ap(), in_=ar1[:, :])
            if n_cores > 1:
                nc.gpsimd.collective_compute(
                    "AllReduce", ALU.add, replica_groups=groups,
                    ins=[cc1_in.ap().opt()], outs=[cc1_out.ap().opt()])
            else:
                nc.sync.dma_start(out=cc1_out.ap(), in_=cc1_in.ap())
            arb1 = res.tile([F2, 4], F32)
            nc.sync.dma_start(out=arb1[:, :], in_=cc1_out.ap())

            def stats_chain(arb, extra_count, ps_sm):
                """arb [128,4] cols (S, Q, negC_or_C, -) -> bcast [128,2] =
                (istd, mu*istd) columns."""
                sums_ps = ps_sm.tile([1, 4], F32, tag="sums")
                nc.tensor.matmul(sums_ps[:, :], ones_col[:, :], arb[:, :],
                                 start=True, stop=True)
                sc = res.tile([1, 12], F32, tag=f"sc{id(arb)}")
                nc.scalar.copy(sc[:, 0:4], sums_ps[:, :])
                # cols: 0=S 1=Q 2=C(+extra) 3=- 4=Cinv 5=mu 6=Q/C 7=mu^2
                #       8=var(+eps) 9=std 10=istd 11=mu*istd
                if extra_count:
                    nc.vector.tensor_scalar(sc[:, 2:3], sc[:, 2:3],
                                            float(extra_count), None, ALU.add)
                nc.vector.reciprocal(sc[:, 4:5], sc[:, 2:3])
                nc.vector.tensor_tensor(sc[:, 5:6], sc[:, 0:1], sc[:, 4:5],
                                        ALU.mult)
                nc.vector.tensor_tensor(sc[:, 6:7], sc[:, 1:2], sc[:, 4:5],
                                        ALU.mult)
                nc.vector.tensor_tensor(sc[:, 7:8], sc[:, 5:6], sc[:, 5:6],
                                        ALU.mult)
                nc.vector.tensor_tensor(sc[:, 8:9], sc[:, 6:7], sc[:, 7:8],
                                        ALU.subtract)
                nc.vector.tensor_scalar(sc[:, 8:9], sc[:, 8:9], EPS, None,
                                        ALU.add)
                nc.scalar.activation(sc[:, 9:10], sc[:, 8:9], AF.Sqrt)
                nc.vector.reciprocal(sc[:, 10:11], sc[:, 9:10])
                nc.vector.tensor_tensor(sc[:, 11:12], sc[:, 5:6],
                                        sc[:, 10:11], ALU.mult)
                bc_ps = ps_sm.tile([128, 2], F32, tag="bc")
                nc.tensor.matmul(bc_ps[:, :], ones_row[:, :], sc[:, 10:12],
                                 start=True, stop=True)
                bc = res.tile([128, 2], F32, tag=f"bc{id(arb)}")
                nc.scalar.copy(bc[:, :], bc_ps[:, :])
                return bc

            with tc.tile_pool(name="ps_sm1", bufs=1, space="PSUM") as ps_sm:
                bc1 = stats_chain(arb1, 0, ps_sm)
            s1c = res.tile([F2, 1], F32)      # istd * gamma
            t1c = res.tile([F2, 1], F32)      # beta - mu * istd * gamma
            nc.vector.tensor_tensor(s1c[:, :], g1c[:, :], bc1[:, 0:1],
                                    ALU.mult)
            nc.vector.tensor_tensor(t1c[:, :], g1c[:, :], bc1[:, 1:2],
                                    ALU.mult)
            nc.vector.tensor_sub(t1c[:, :], b1c[:, :], t1c[:, :])
            # duplicated (stacked-half) forms
            sf = res.tile([128, 1], F32)
            tf = res.tile([128, 1], F32)
            sg = res.tile([128, 1], F32)
            tg = res.tile([128, 1], F32)
            for dst, src in ((sf, s1c), (tf, t1c)):
                nc.vector.tensor_copy(dst[0:64, :], src[0:64, :])
                nc.vector.tensor_copy(dst[64:128, :], src[0:64, :])
            for dst, src in ((sg, s1c), (tg, t1c)):
                nc.vector.tensor_copy(dst[0:64, :], src[64:128, :])
                nc.vector.tensor_copy(dst[64:128, :], src[64:128, :])

            if stage == 3:
                nc.sync.dma_start(out=out_d[0:F2, 0:1], in_=s1c[:, :])
                return

            # ============ pass 2 ============
            ns2 = res.tile([128, NOD2], F32)
            with tc.tile_pool(name="p2", bufs=2) as p2:
                for c in range(HALF // CW):
                    s0 = slice(c * CW, (c + 1) * CW)
                    s1 = slice(HALF + c * CW, HALF + (c + 1) * CW)
                    wf = p2.tile([128, CW], F16, tag="wf")
                    nc.vector.tensor_copy(wf[0:64, :], w_all[0:F, s0])
                    nc.vector.tensor_copy(wf[64:128, :], w_all[0:F, s1])
                    wc = p2.tile([128, CW], F16, tag="wc")
                    nc.vector.tensor_copy(wc[0:64, :], w_all[F:F2, s0])
                    nc.vector.tensor_copy(wc[64:128, :], w_all[F:F2, s1])
                    gf = p2.tile([128, CW], F16, tag="gf")
                    nc.vector.memset(gf[:, :], SIG0)
                    sig = p2.tile([128, CW], F16, tag="sig")
                    nc.scalar.activation(sig[:, :], wf[:, :], AF.Sigmoid,
                                         bias=tf[:, :], scale=sf[:, :])
                    pif = p2.tile([128, CW], I8, tag="pif")
                    nc.vector.tensor_scalar(pif[:, :], wf[:, :], 0.0, None,
                                            ALU.not_equal)
                    nc.vector.copy_predicated(gf[:, :], pif[:, :], sig[:, :])
                    gc_ = p2.tile([128, CW], F16, tag="gc")
                    nc.vector.memset(gc_[:, :], LN2)
                    ex = p2.tile([128, CW], F32, tag="ex")
                    nc.scalar.activation(ex[:, :], wc[:, :], AF.Exp,
                                         bias=tg[:, :], scale=sg[:, :])
                    sof = p2.tile([128, CW], F16, tag="sof")
                    nc.scalar.activation(sof[:, :], ex[:, :], AF.Ln, bias=1.0)
                    pic = p2.tile([128, CW], I8, tag="pic")
                    nc.vector.tensor_scalar(pic[:, :], wc[:, :], 0.0, None,
                                            ALU.not_equal)
                    nc.vector.copy_predicated(gc_[:, :], pic[:, :], sof[:, :])
                    gtd = p2.tile([128, CW], F16, tag="gtd")
                    nc.vector.tensor_mul(gtd[:, :], gf[:, :], gc_[:, :])
                    nsr = p2.tile([128, CW // M], F32, tag="nsr")
                    nc.vector.tensor_reduce(
                        nsr[:, :],
                        gtd.rearrange("p (n m) -> p n m", m=M),
                        axis=mybir.AxisListType.X, op=ALU.add)
                    nsl = slice(c * (CW // M), (c + 1) * (CW // M))
                    pim = p2.tile([128, CW // M], I8, tag="pim")
                    nc.vector.tensor_scalar(pim[:, :], mnode2[:, nsl], 0.0,
                                            None, ALU.not_equal)
                    nc.vector.memset(ns2[:, nsl], 0.0)
                    nc.vector.copy_predicated(ns2[:, nsl], pim[:, :],
                                              nsr[:, :])

            if debug:
                with tc.tile_pool(name="dbgw", bufs=1) as dbgp:
                    for j in range(RR // 512):
                        dw = dbgp.tile([F2, 512], F32, tag="dw")
                        nc.vector.tensor_copy(dw[:, :], w_all[:, j * 512:(j + 1) * 512])
                        nc.sync.dma_start(out=dbg_w[:, j * 512:(j + 1) * 512], in_=dw[:, :])
                    nc.sync.dma_start(out=dbg_ar1, in_=arb1[:, :])
                    dst = dbgp.tile([F2, 4], F32, tag="dst")
                    nc.vector.tensor_copy(dst[:, 0:1], s1c[:, :])
                    nc.vector.tensor_copy(dst[:, 1:2], t1c[:, :])
                    nc.vector.tensor_copy(dst[:, 2:3], sf[:, :])
                    nc.vector.tensor_copy(dst[:, 3:4], tf[:, :])
                    nc.sync.dma_start(out=dbg_st, in_=dst[:, :])

            if stage == 4:
                nc.sync.dma_start(out=out_d[0:F2, 0:F], in_=ns2[:, 0:F])
                return

            # ============ stats 2 + AllReduce #2 ============
            ar2 = res.tile([F2, 4], F32)
            nc.vector.tensor_reduce(ar2[:, 0:1], ns2[:, :],
                                    axis=mybir.AxisListType.X, op=ALU.add)
            with tc.tile_pool(name="q2s", bufs=1) as q2s:
                q2scr = q2s.tile([128, NOD2], F16)
                nc.scalar.activation(q2scr[:, :], ns2[:, :], AF.Square,
                                     accum_out=ar2[:, 1:2])
                eqt = q2s.tile([128, NOD2], F16)
                nc.vector.tensor_scalar(eqt[:, :], ns2[:, :], 0.0, None,
                                        ALU.is_equal)
                nc.vector.tensor_reduce(ar2[:, 2:3], eqt[:, :],
                                        axis=mybir.AxisListType.X,
                                        op=ALU.add, negate=True)
            nc.vector.memset(ar2[:, 3:4], 0.0)
            nc.sync.dma_start(out=cc2_in.ap(), in_=ar2[:, :])
            if n_cores > 1:
                nc.gpsimd.collective_compute(
                    "AllReduce", ALU.add, replica_groups=groups,
                    ins=[cc2_in.ap().opt()], outs=[cc2_out.ap().opt()])
            else:
                nc.sync.dma_start(out=cc2_out.ap(), in_=cc2_in.ap())
            arb2 = res.tile([F2, 4], F32)
            nc.sync.dma_start(out=arb2[:, :], in_=cc2_out.ap())
            with tc.tile_pool(name="ps_sm2", bufs=1, space="PSUM") as ps_sm:
                bc2 = stats_chain(arb2, C2CONST, ps_sm)
            if debug:
                nc.sync.dma_start(out=dbg_ar2, in_=arb2[:, :])
                nc.sync.dma_start(out=dbg_ns2, in_=ns2[:, :])
            s2c = res.tile([F, 1], F32)
            t2c = res.tile([F, 1], F32)
            nc.vector.tensor_tensor(s2c[:, :], g2c[:, :], bc2[0:F, 0:1],
                                    ALU.mult)
            nc.vector.tensor_tensor(t2c[:, :], g2c[:, :], bc2[0:F, 1:2],
                                    ALU.mult)
            nc.vector.tensor_sub(t2c[:, :], b2c[:, :], t2c[:, :])

            # ============ pass 3 ============
            CP = 512 if (RN // 2) % 512 == 0 else RN // 2
            TS = min(128, CP)
            with (
                tc.tile_pool(name="p3", bufs=2) as p3,
                tc.tile_pool(name="ops", bufs=2, space="PSUM") as ops,
            ):
                for c in range(RN // CP):
                    cs = slice(c * CP, (c + 1) * CP)
                    half, col = divmod(c * CP, RN // 2)
                    nsl = slice(half * 64, half * 64 + 64)
                    ns_c = p3.tile([F, CP], F32, tag="ns")
                    nc.vector.tensor_copy(ns_c[:, :],
                                          ns2[nsl, col:col + CP])
                    y2 = p3.tile([F, CP], F32, tag="y2")
                    nc.vector.tensor_scalar(y2[:, :], ns_c[:, :], s2c[:, :],
                                            t2c[:, :], ALU.mult, ALU.add)
                    nc.vector.tensor_add(y2[:, :], y2[:, :], atomT32[:, cs])
                    ex3 = p3.tile([F, CP], F32, tag="ex3")
                    nc.scalar.activation(ex3[:, :], y2[:, :], AF.Exp)
                    sp = p3.tile([F, CP], F32, tag="sp")
                    nc.scalar.activation(sp[:, :], ex3[:, :], AF.Ln, bias=1.0)
                    fin = p3.tile([F, CP], F32, tag="fin")
                    nc.vector.memset(fin[:, :], 0.0)
                    pin = p3.tile([F, CP], I8, tag="pin")
                    nc.vector.tensor_scalar(pin[:, :], ns_c[:, :], 0.0, None,
                                            ALU.not_equal)
                    nc.vector.copy_predicated(fin[:, :], pin[:, :], sp[:, :])
                    for j in range(CP // TS):
                        op = ops.tile([TS, F], F32)
                        nc.tensor.transpose(op[:, :],
                                            fin[:, j * TS:(j + 1) * TS],
                                            ident32[0:F, 0:F])
                        ob = p3.tile([TS, F], F32, tag="ob")
                        nc.scalar.copy(ob[:, :], op[:, :])
                        r0 = c * CP + j * TS
                        nc.sync.dma_start(out=out_d[r0:r0 + TS, :],
                                          in_=ob[:, :])


def make_in_maps(inputs, b_loc, n_cores):
    """Shard full inputs over cores (batch-split); replicate weights."""
    atom = np.ascontiguousarray(inputs["atom_fea"], dtype=np.float32)
    nbr = np.ascontiguousarray(inputs["nbr_fea"], dtype=np.float32)
    idx = np.ascontiguousarray(inputs["nbr_fea_idx"], dtype=np.int32)
    mask = np.ascontiguousarray(inputs["mask"], dtype=np.int32)
    w = np.ascontiguousarray(inputs["gc_W"], dtype=np.float32)
    bias = np.ascontiguousarray(inputs["gc_bias"], dtype=np.float32)
    g1 = np.ascontiguousarray(inputs["gamma_1"], dtype=np.float32)
    b1 = np.ascontiguousarray(inputs["beta_1"], dtype=np.float32)
    g2 = np.ascontiguousarray(inputs["gamma_2"], dtype=np.float32)
    b2 = np.ascontiguousarray(inputs["beta_2"], dtype=np.float32)
    in_maps = []
    for i in range(n_cores):
        bs = slice(i * b_loc, (i + 1) * b_loc)
        in_maps.append({
            "atom": atom[bs].reshape(b_loc * N, F),
            "nbr": nbr[bs].reshape(b_loc * N * M, NF),
            "idx": idx[bs].reshape(b_loc * N, M),
            "mask": mask[bs].reshape(b_loc * N * M, F2),
            "gcw": w,
            "gcb": bias.reshape(1, F2),
            "g1": g1.reshape(F2, 1),
            "b1": b1.reshape(F2, 1),
            "g2": g2.reshape(F, 1),
            "b2": b2.reshape(F, 1),
        })
    return in_maps


_CACHED = {}


def _get_nc(b_loc, n_cores):
    key = (b_loc, n_cores)
    if key not in _CACHED:
        nc = bacc.Bacc("TRN2", target_bir_lowering=False, debug=False,
                       num_devices=n_cores)
        build(nc, b_loc, n_cores)
        nc.compile()
        _CACHED[key] = nc
    return _CACHED[key]


LAST_RESULTS = None


def kernel(**inputs) -> np.ndarray:
    import os
    from concourse.bass_utils import run_bass_kernel_spmd

    global LAST_RESULTS
    n_cores = 8
    b_loc = B // n_cores
    nc = _get_nc(b_loc, n_cores)
    in_maps = make_in_maps(inputs, b_loc, n_cores)
    trace = os.environ.get("CGC_TRACE") == "1"
    res = run_bass_kernel_spmd(nc, in_maps, core_ids=list(range(n_cores)),
                               trace=trace)
    LAST_RESULTS = res
    out = np.concatenate(
        [res.results[i]["out"].reshape(b_loc, N, F) for i in range(n_cores)],
        axis=0)
    return out.astype(np.float32, copy=False)
